# revision 18
# baseline (speedup 1.0000x reference)
"""Trainium2 Bass kernel for multi-head attention (B=4, T=2048, HID=1024, H=16, D=64).

Sharding (8 NeuronCores): core c owns batch b = c//2 and query rows
g = c%2 (1024 of 2048). No collectives: both cores of a batch pair
redundantly project the batch's full key/value set, which is far cheaper
under this machine's interconnect than any inter-core exchange.

Mask semantics: the reference tiles the pad mask head-major
(jnp.tile(pad_mask, (H, 1)) against batch-major split heads), so head h
attends under mask row pad_mask[h % 4] for EVERY batch. The kernel takes
a per-class additive-bias input bias[4, T] (with a -3 shift folded in to
keep exp() small; softmax is shift-invariant).

Host staging (kernel() below): activations/weights pre-transposed into
PE-ready layouts and cast to bf16 (zero device-side input transposes),
Wq pre-scaled by D**-0.5, Wq/Wk/Wv pre-sliced per head-pair so weight
slices stream through a small rotating pool.

Device pipeline per core: per head-pair projections (kp -> [j,t],
vp -> [t,j] with an appended ones column for the softmax
denominator, qp -> [j,t]) feed head-pipelined attention: scores st[k,q]
on PE, probabilities via one Scalar-engine exp per key tile (bias folds
the mask), then PV with P chunks as the stationary operand producing ctx[q, d+1] per query tile in its own PSUM bank
(kt-inner accumulation: matmul start=True clobbers bank-wide, so each
accumulator owns a bank and sees exactly one start). PV lags scores by
one head so exp latency hides. Per-partition softmax normalize on DVE.
A tail PE-transpose of ctx feeds the row-parallel output projection,
which tiles the full output with no reduction.
"""

from contextlib import ExitStack

import numpy as np

import concourse.bacc as bacc
import concourse.mybir as mybir
import concourse.tile as tile
from concourse.masks import make_identity

F32 = mybir.dt.float32
BF16 = mybir.dt.bfloat16
EXP = mybir.ActivationFunctionType.Exp

B, T, HID, H, D = 4, 2048, 1024, 16, 64
TQ = T // 2            # query rows owned by one core
KT = T // 128          # 16 key tiles
QT = TQ // 128         # 8 query tiles
IO = HID // 128        # 8 contraction blocks
HP = H // 2            # 8 head pairs
NCLS = 4               # pad-mask classes (head h uses class h % 4)
N_CORES = 8
NEG_INF = -1.0e9
BIAS_SHIFT = -3.0      # keeps exp() outputs order-1


def _emit(tc, qT_d, kT_d, vT_d, bias_d, wqs_d, wks_d, wvs_d, wo_d, out_d):
    nc = tc.nc
    with ExitStack() as ctx:
        const = ctx.enter_context(tc.tile_pool(name="const", bufs=1))
        ident = const.tile([128, 128], BF16)
        make_identity(nc, ident)
        bias_sb = const.tile([128, NCLS, KT], F32)
        nc.sync.dma_start(
            bias_sb[:], bias_d.ap().rearrange("c (kt p) -> p c kt", p=128))
        ctxN = const.tile([128, QT, HID], BF16)   # [q%128, qt, j] normalized ctx

        big = ctx.enter_context(
            tc.tile_pool(name="big", bufs=3, space="PSUM"))

        with tc.tile_pool(name="wsl", bufs=2) as wsp, \
             tc.tile_pool(name="xin", bufs=1) as xip, \
             tc.tile_pool(name="kpp", bufs=2) as kpp, \
             tc.tile_pool(name="qpp", bufs=2) as qpp, \
             tc.tile_pool(name="vpp", bufs=3) as vpp, \
             tc.tile_pool(name="pt", bufs=34) as ptp, \
             tc.tile_pool(name="rden", bufs=4) as rdp, \
             tc.tile_pool(name="ctxps", bufs=2, space="PSUM") as cxp:

            def dma_w(hp):
                # per-head-pair weight slices, host-staged contiguous
                wk = wsp.tile([128, IO, 128], BF16, tag="wk")
                nc.sync.dma_start(wk[:], wks_d.ap()[hp])
                wq = wsp.tile([128, IO, 128], BF16, tag="wq")
                nc.sync.dma_start(wq[:], wqs_d.ap()[hp])
                wv = wsp.tile([128, IO, 128], BF16, tag="wv")
                nc.sync.dma_start(wv[:], wvs_d.ap()[hp])
                return wk, wq, wv

            # ---- input DMAs: first weight slices and the first kT half
            # lead, so the first projection starts ~7us in ----
            ws = {0: dma_w(0)}
            kT_sb = xip.tile([128, IO, T], BF16, tag="kT")
            kT_src = kT_d.ap().rearrange("(io p) t -> p io t", p=128)
            nc.sync.dma_start(kT_sb[:, :, 0:512], kT_src[:, :, 0:512])
            nc.sync.dma_start(kT_sb[:, :, 512:1024], kT_src[:, :, 512:1024])
            qT_sb = xip.tile([128, IO, TQ], BF16, tag="qT")
            nc.sync.dma_start(qT_sb[:, :, 0:512], qT_d.ap().rearrange(
                "(io p) t -> p io t", p=128)[:, :, 0:512])
            nc.sync.dma_start(qT_sb[:, :, 512:1024], qT_d.ap().rearrange(
                "(io p) t -> p io t", p=128)[:, :, 512:1024])
            nc.sync.dma_start(kT_sb[:, :, 1024:T], kT_src[:, :, 1024:T])
            ws[1] = dma_w(1)
            vT_sb = xip.tile([128, IO, T], BF16, tag="vT")
            vT_src = vT_d.ap().rearrange("(io p) t -> p io t", p=128)
            nc.sync.dma_start(vT_sb[:, :, 0:512], vT_src[:, :, 0:512])
            nc.sync.dma_start(vT_sb[:, :, 512:1024], vT_src[:, :, 512:1024])
            ws[2] = dma_w(2)
            nc.sync.dma_start(vT_sb[:, :, 1024:T], vT_src[:, :, 1024:T])

            def make_proj(hp, wk, wq, wv):
                """Allocate the pair's projection tiles; return (tiles, gen).

                The generator emits the projection matmuls in ~0.5-1.7us
                chunks so the driver can interleave them between score
                tiles, keeping PE fed while the Scalar engine drains exps.
                """
                kpTt = kpp.tile([128, T], BF16, tag="kpT")
                qpTt = qpp.tile([128, TQ], BF16, tag="qpT")
                vpmt = vpp.tile([128, KT, 2, 65], BF16, tag="vpm")
                nc.gpsimd.memset(vpmt[:, :, :, 64:65], 1.0)

                def gen():
                    for tg in range(2):
                        ps = big.tile([128, 1024], F32, tag="big")
                        for half in range(2):
                            for io in range(IO):
                                nc.tensor.matmul(
                                    ps[:, half * 512:(half + 1) * 512],
                                    wk[:, io, :],
                                    kT_sb[:, io, tg * 1024 + half * 512:
                                          tg * 1024 + (half + 1) * 512],
                                    start=(io == 0), stop=(io == IO - 1))
                            if half == 1:
                                nc.vector.tensor_copy(
                                    kpTt[:, tg * 1024:(tg + 1) * 1024], ps[:])
                            yield
                    ps = big.tile([128, 1024], F32, tag="big")
                    for half in range(2):
                        for io in range(IO):
                            nc.tensor.matmul(
                                ps[:, half * 512:(half + 1) * 512],
                                wq[:, io, :],
                                qT_sb[:, io, half * 512:(half + 1) * 512],
                                start=(io == 0), stop=(io == IO - 1))
                        if half == 1:
                            nc.vector.tensor_copy(qpTt[:], ps[:])
                        yield
                    for tg in range(2):
                        ps = big.tile([128, 1024], F32, tag="big")
                        for tt8 in range(8):
                            tt = tg * 8 + tt8
                            for io in range(IO):
                                nc.tensor.matmul(
                                    ps[:, tt8 * 128:(tt8 + 1) * 128],
                                    vT_sb[:, io, tt * 128:(tt + 1) * 128],
                                    wv[:, io, :],
                                    start=(io == 0), stop=(io == IO - 1))
                            if tt8 == 7:
                                nc.vector.tensor_copy(
                                    vpmt[:, tg * 8:(tg + 1) * 8, :, 0:64],
                                    ps[:].rearrange("p (tt hh d) -> p tt hh d",
                                                    tt=8, hh=2))
                            if tt8 % 2 == 1:
                                yield

                return (kpTt, qpTt, vpmt), gen()

            N_PROJ_PULLS = 14  # yields per proj generator (4 kp + 2 qp + 8 vp)

            def attn_tick(h, kpTt, qpTt, kt):
                # one score tile + its exp; returns the resident P tile
                r, c = h % 2, h % NCLS
                st = big.tile([128, 1024], F32, tag="big")
                for half in range(2):
                    nc.tensor.matmul(
                        st[:, half * 512:(half + 1) * 512],
                        kpTt[r * 64:(r + 1) * 64, kt * 128:(kt + 1) * 128],
                        qpTt[r * 64:(r + 1) * 64, half * 512:(half + 1) * 512],
                        start=True, stop=True,
                        tile_position=(r * 64, 0))
                pt = ptp.tile([128, 1024], BF16, tag="pt", bufs=34)
                nc.scalar.activation(pt[:], st[:], EXP,
                                     bias=bias_sb[:, c, kt:kt + 1])
                return pt

            def pv_gen(h, pts, vpmt):
                # kt-inner PV: each qt accumulator owns one PSUM bank, so it
                # sees exactly one start=True (start clobbers bank-wide)
                r = h % 2
                for qt in range(QT):
                    cx = cxp.tile([128, 128], F32, tag="cx")
                    for kt in range(KT):
                        nc.tensor.matmul(
                            cx[:, 0:65],
                            pts[kt][:, qt * 128:(qt + 1) * 128],
                            vpmt[:, kt, r, :],
                            start=(kt == 0), stop=(kt == KT - 1))
                    rden = rdp.tile([128, 1], F32, tag="rden")
                    nc.vector.reciprocal(rden[:], cx[:, 64:65])
                    nc.vector.tensor_scalar_mul(
                        ctxN[:, qt, h * 64:(h + 1) * 64],
                        cx[:, 0:64], rden[:])
                    yield

            # ---- pipelined emission: weight DMAs 2 stages ahead, projection
            # chunks interleaved between score tiles one stage ahead, PV one
            # head behind scores (also interleaved) ----
            tiles0, pg0 = make_proj(0, *ws[0])
            for _ in pg0:
                pass
            projs = {0: tiles0}
            pend = None
            pg = None
            for hp in range(HP):
                if hp + 2 < HP and hp + 2 not in ws and hp + 2 > 2:
                    ws[hp + 2] = dma_w(hp + 2)
                if hp + 1 < HP:
                    projs[hp + 1], pg = make_proj(hp + 1, *ws[hp + 1])
                    ws.pop(hp + 1)
                else:
                    pg = None
                kpTt, qpTt, vpmt = projs.pop(hp)
                pulled = 0
                pvg = None
                for r in range(2):
                    h = 2 * hp + r
                    pvg = pv_gen(*pend) if pend is not None else None
                    pts = []
                    for kt in range(KT):
                        pts.append(attn_tick(h, kpTt, qpTt, kt))
                        if pvg is not None and kt % 2 == 1:
                            next(pvg, None)
                        tick = r * KT + kt + 1
                        want = (tick * N_PROJ_PULLS) // (2 * KT)
                        while pg is not None and pulled < want:
                            if next(pg, StopIteration) is StopIteration:
                                pg = None
                                break
                            pulled += 1
                    pend = (h, pts, vpmt)
            for _ in pv_gen(*pend):
                pass

        # ---------------- tail: transpose ctx + output projection ----------
        with tc.tile_pool(name="ctxT", bufs=1) as ctp, \
             tc.tile_pool(name="ostage", bufs=2) as osp, \
             tc.tile_pool(name="tailps", bufs=2, space="PSUM") as tlp:
            # Wo loads into SBUF freed by the projection pools; the DMA can
            # start as soon as the last projection read retires
            wo_sb = ctp.tile([128, IO, HID], BF16, tag="wo")
            nc.sync.dma_start(wo_sb[:], wo_d.ap().rearrange("(io p) j -> p io j", p=128))
            ctxT = ctp.tile([128, IO, TQ], BF16)
            for jb in range(IO):
                ps = tlp.tile([128, 1024], BF16, tag="tpose")
                for qt in range(QT):
                    nc.tensor.matmul(
                        ps[:, qt * 128:(qt + 1) * 128],
                        ctxN[:, qt, jb * 128:(jb + 1) * 128], ident[:],
                        is_transpose=True, start=True, stop=True)
                nc.vector.tensor_copy(ctxT[:, jb, :], ps[:])
            for tt in range(QT):
                pso = big.tile([128, 1024], F32, tag="big")
                for half in range(2):
                    for jb in range(IO):
                        nc.tensor.matmul(
                            pso[:, half * 512:(half + 1) * 512],
                            ctxT[:, jb, tt * 128:(tt + 1) * 128],
                            wo_sb[:, jb, half * 512:(half + 1) * 512],
                            start=(jb == 0), stop=(jb == IO - 1))
                ost = osp.tile([128, 1024], F32, tag="ost")
                nc.vector.tensor_copy(ost[:], pso[:])
                nc.sync.dma_start(out_d.ap()[tt * 128:(tt + 1) * 128, :], ost[:])


def build():
    nc = bacc.Bacc("TRN2", target_bir_lowering=False, debug=False,
                   num_devices=N_CORES)
    qT_d = nc.dram_tensor("qT", [HID, TQ], BF16, kind="ExternalInput")
    kT_d = nc.dram_tensor("kT", [HID, T], BF16, kind="ExternalInput")
    vT_d = nc.dram_tensor("vT", [HID, T], BF16, kind="ExternalInput")
    bias_d = nc.dram_tensor("bias", [NCLS, T], F32, kind="ExternalInput")
    wqs_d = nc.dram_tensor("Wqs", [HP, 128, IO, 128], BF16, kind="ExternalInput")
    wks_d = nc.dram_tensor("Wks", [HP, 128, IO, 128], BF16, kind="ExternalInput")
    wvs_d = nc.dram_tensor("Wvs", [HP, 128, IO, 128], BF16, kind="ExternalInput")
    wo_d = nc.dram_tensor("Wo", [HID, HID], BF16, kind="ExternalInput")
    out_d = nc.dram_tensor("out", [TQ, HID], F32, kind="ExternalOutput")

    with tile.TileContext(nc) as tc:
        _emit(tc, qT_d, kT_d, vT_d, bias_d, wqs_d, wks_d, wvs_d, wo_d, out_d)
    nc.compile()
    return nc


_NC = None


def _get_nc():
    global _NC
    if _NC is None:
        _NC = build()
    return _NC


def _slice_weight(w):
    # [HID, HID] -> [HP, 128, IO, 128]: staged[hp, p, io, jj] = w[io*128+p, hp*128+jj]
    return np.ascontiguousarray(
        w.reshape(IO, 128, HP, 128).transpose(2, 1, 0, 3))


def kernel(**inputs):
    import ml_dtypes
    from concourse.bass_utils import run_bass_kernel_spmd

    bf16 = ml_dtypes.bfloat16
    q = np.asarray(inputs["q"], dtype=np.float32)
    k = np.asarray(inputs["k"], dtype=np.float32)
    v = np.asarray(inputs["v"], dtype=np.float32)
    pm = np.asarray(inputs["pad_mask"], dtype=np.float32)
    wqs = _slice_weight(
        (np.asarray(inputs["Wq"], dtype=np.float32) * (D ** -0.5)).astype(bf16))
    wks = _slice_weight(np.asarray(inputs["Wk"], dtype=np.float32).astype(bf16))
    wvs = _slice_weight(np.asarray(inputs["Wv"], dtype=np.float32).astype(bf16))
    wo = np.asarray(inputs["Wo"], dtype=np.float32).astype(bf16)

    # head h is masked by pad_mask[h % 4] (reference tiles the mask
    # head-major); shift keeps exp() inside fp8e4m3 range
    bias = (pm[0:NCLS] * NEG_INF + BIAS_SHIFT).astype(np.float32)

    kTs = [np.ascontiguousarray(k[b].T.astype(bf16)) for b in range(B)]
    vTs = [np.ascontiguousarray(v[b].T.astype(bf16)) for b in range(B)]

    in_maps = []
    for c in range(N_CORES):
        b, g = c // 2, c % 2
        qT = np.ascontiguousarray(q[b, g * TQ:(g + 1) * TQ, :].T.astype(bf16))
        in_maps.append({
            "qT": qT,
            "kT": kTs[b],
            "vT": vTs[b],
            "bias": bias,
            "Wqs": wqs, "Wks": wks, "Wvs": wvs, "Wo": wo,
        })
    res = run_bass_kernel_spmd(_get_nc(), in_maps, list(range(N_CORES))).results
    out = np.empty((B, T, HID), np.float32)
    for c in range(N_CORES):
        b, g = c // 2, c % 2
        out[b, g * TQ:(g + 1) * TQ] = res[c]["out"]
    return out


# revision 19
# speedup vs baseline: 1.0175x; 1.0175x over previous
"""Trainium2 Bass kernel for multi-head attention (B=4, T=2048, HID=1024, H=16, D=64).

Sharding (8 NeuronCores): core c owns batch b = c//2 and query rows
g = c%2 (1024 of 2048). No collectives: both cores of a batch pair
redundantly project the batch's full key/value set, which is far cheaper
under this machine's interconnect than any inter-core exchange.

Mask semantics: the reference tiles the pad mask head-major
(jnp.tile(pad_mask, (H, 1)) against batch-major split heads), so head h
attends under mask row pad_mask[h % 4] for EVERY batch. The kernel takes
a per-class additive-bias input bias[4, T] (with a -3 shift folded in to
keep exp() small; softmax is shift-invariant).

Host staging (kernel() below): activations/weights pre-transposed into
PE-ready layouts and cast to bf16 (zero device-side input transposes),
Wq pre-scaled by D**-0.5, Wq/Wk/Wv pre-sliced per head-pair so weight
slices stream through a small rotating pool.

Device pipeline per core: per head-pair projections (kp -> [j,t],
vp -> [t,j] with an appended ones column for the softmax
denominator, qp -> [j,t]) feed head-pipelined attention: scores st[k,q]
on PE, probabilities via one Scalar-engine exp per key tile (bias folds
the mask), then PV with P chunks as the stationary operand producing ctx[q, d+1] per query tile in its own PSUM bank
(kt-inner accumulation: matmul start=True clobbers bank-wide, so each
accumulator owns a bank and sees exactly one start). PV lags scores by
one head so exp latency hides. Per-partition softmax normalize on DVE.
A tail PE-transpose of ctx feeds the row-parallel output projection,
which tiles the full output with no reduction.
"""

from contextlib import ExitStack

import numpy as np

import concourse.bacc as bacc
import concourse.mybir as mybir
import concourse.tile as tile
from concourse.masks import make_identity

F32 = mybir.dt.float32
BF16 = mybir.dt.bfloat16
EXP = mybir.ActivationFunctionType.Exp

B, T, HID, H, D = 4, 2048, 1024, 16, 64
TQ = T // 2            # query rows owned by one core
KT = T // 128          # 16 key tiles
QT = TQ // 128         # 8 query tiles
IO = HID // 128        # 8 contraction blocks
HP = H // 2            # 8 head pairs
NCLS = 4               # pad-mask classes (head h uses class h % 4)
N_CORES = 8
NEG_INF = -1.0e9
BIAS_SHIFT = -3.0      # keeps exp() outputs order-1


def _emit(tc, qT_d, kT_d, vT_d, bias_d, wqs_d, wks_d, wvs_d, wo_d, out_d):
    nc = tc.nc
    with ExitStack() as ctx:
        const = ctx.enter_context(tc.tile_pool(name="const", bufs=1))
        ident = const.tile([128, 128], BF16)
        make_identity(nc, ident)
        bias_sb = const.tile([128, NCLS, KT], F32)
        nc.sync.dma_start(
            bias_sb[:], bias_d.ap().rearrange("c (kt p) -> p c kt", p=128))
        ctxN = const.tile([128, QT, HID], BF16)   # [q%128, qt, j] normalized ctx

        big = ctx.enter_context(
            tc.tile_pool(name="big", bufs=3, space="PSUM"))

        with tc.tile_pool(name="wsl", bufs=2) as wsp, \
             tc.tile_pool(name="xin", bufs=1) as xip, \
             tc.tile_pool(name="kpp", bufs=2) as kpp, \
             tc.tile_pool(name="qpp", bufs=2) as qpp, \
             tc.tile_pool(name="vpp", bufs=3) as vpp, \
             tc.tile_pool(name="pt", bufs=34) as ptp, \
             tc.tile_pool(name="rden", bufs=4) as rdp, \
             tc.tile_pool(name="ctxps", bufs=2, space="PSUM") as cxp:

            def dma_w(hp):
                # per-head-pair weight slices, host-staged contiguous
                wk = wsp.tile([128, IO, 128], BF16, tag="wk")
                nc.sync.dma_start(wk[:], wks_d.ap()[hp])
                wq = wsp.tile([128, IO, 128], BF16, tag="wq")
                nc.sync.dma_start(wq[:], wqs_d.ap()[hp])
                wv = wsp.tile([128, IO, 128], BF16, tag="wv")
                nc.sync.dma_start(wv[:], wvs_d.ap()[hp])
                return wk, wq, wv

            # ---- input DMAs: first weight slices and the first kT half
            # lead, so the first projection starts ~7us in ----
            ws = {0: dma_w(0)}
            kT_sb = xip.tile([128, IO, T], BF16, tag="kT")
            kT_src = kT_d.ap().rearrange("(io p) t -> p io t", p=128)
            nc.sync.dma_start(kT_sb[:, :, 0:1024], kT_src[:, :, 0:1024])
            qT_sb = xip.tile([128, IO, TQ], BF16, tag="qT")
            nc.sync.dma_start(qT_sb[:], qT_d.ap().rearrange("(io p) t -> p io t", p=128))
            nc.sync.dma_start(kT_sb[:, :, 1024:T], kT_src[:, :, 1024:T])
            ws[1] = dma_w(1)
            vT_sb = xip.tile([128, IO, T], BF16, tag="vT")
            vT_src = vT_d.ap().rearrange("(io p) t -> p io t", p=128)
            nc.sync.dma_start(vT_sb[:, :, 0:1024], vT_src[:, :, 0:1024])
            ws[2] = dma_w(2)
            nc.sync.dma_start(vT_sb[:, :, 1024:T], vT_src[:, :, 1024:T])

            def make_proj(hp, wk, wq, wv):
                """Allocate the pair's projection tiles; return (tiles, gen).

                The generator emits the projection matmuls in ~0.5-1.7us
                chunks so the driver can interleave them between score
                tiles, keeping PE fed while the Scalar engine drains exps.
                """
                kpTt = kpp.tile([128, T], BF16, tag="kpT")
                qpTt = qpp.tile([128, TQ], BF16, tag="qpT")
                vpmt = vpp.tile([128, KT, 2, 65], BF16, tag="vpm")
                nc.gpsimd.memset(vpmt[:, :, :, 64:65], 1.0)

                def gen():
                    for tg in range(2):
                        ps = big.tile([128, 1024], F32, tag="big")
                        for half in range(2):
                            for io in range(IO):
                                nc.tensor.matmul(
                                    ps[:, half * 512:(half + 1) * 512],
                                    wk[:, io, :],
                                    kT_sb[:, io, tg * 1024 + half * 512:
                                          tg * 1024 + (half + 1) * 512],
                                    start=(io == 0), stop=(io == IO - 1))
                            if half == 1:
                                nc.vector.tensor_copy(
                                    kpTt[:, tg * 1024:(tg + 1) * 1024], ps[:])
                            yield
                    ps = big.tile([128, 1024], F32, tag="big")
                    for half in range(2):
                        for io in range(IO):
                            nc.tensor.matmul(
                                ps[:, half * 512:(half + 1) * 512],
                                wq[:, io, :],
                                qT_sb[:, io, half * 512:(half + 1) * 512],
                                start=(io == 0), stop=(io == IO - 1))
                        if half == 1:
                            nc.vector.tensor_copy(qpTt[:], ps[:])
                        yield
                    for tg in range(2):
                        ps = big.tile([128, 1024], F32, tag="big")
                        for tt8 in range(8):
                            tt = tg * 8 + tt8
                            for io in range(IO):
                                nc.tensor.matmul(
                                    ps[:, tt8 * 128:(tt8 + 1) * 128],
                                    vT_sb[:, io, tt * 128:(tt + 1) * 128],
                                    wv[:, io, :],
                                    start=(io == 0), stop=(io == IO - 1))
                            if tt8 == 7:
                                nc.vector.tensor_copy(
                                    vpmt[:, tg * 8:(tg + 1) * 8, :, 0:64],
                                    ps[:].rearrange("p (tt hh d) -> p tt hh d",
                                                    tt=8, hh=2))
                            if tt8 % 2 == 1:
                                yield

                return (kpTt, qpTt, vpmt), gen()

            N_PROJ_PULLS = 14  # yields per proj generator (4 kp + 2 qp + 8 vp)

            def attn_tick(h, kpTt, qpTt, kt):
                # one score tile + its exp; returns the resident P tile
                r, c = h % 2, h % NCLS
                st = big.tile([128, 1024], F32, tag="big")
                for half in range(2):
                    nc.tensor.matmul(
                        st[:, half * 512:(half + 1) * 512],
                        kpTt[r * 64:(r + 1) * 64, kt * 128:(kt + 1) * 128],
                        qpTt[r * 64:(r + 1) * 64, half * 512:(half + 1) * 512],
                        start=True, stop=True,
                        tile_position=(r * 64, 0))
                pt = ptp.tile([128, 1024], BF16, tag="pt", bufs=34)
                nc.scalar.activation(pt[:], st[:], EXP,
                                     bias=bias_sb[:, c, kt:kt + 1])
                return pt

            def pv_gen(h, pts, vpmt):
                # kt-inner PV: each qt accumulator owns one PSUM bank, so it
                # sees exactly one start=True (start clobbers bank-wide)
                r = h % 2
                for qt in range(QT):
                    cx = cxp.tile([128, 128], F32, tag="cx")
                    for kt in range(KT):
                        nc.tensor.matmul(
                            cx[:, 0:65],
                            pts[kt][:, qt * 128:(qt + 1) * 128],
                            vpmt[:, kt, r, :],
                            start=(kt == 0), stop=(kt == KT - 1))
                    rden = rdp.tile([128, 1], F32, tag="rden")
                    nc.vector.reciprocal(rden[:], cx[:, 64:65])
                    nc.vector.tensor_scalar_mul(
                        ctxN[:, qt, h * 64:(h + 1) * 64],
                        cx[:, 0:64], rden[:])
                    yield

            # ---- pipelined emission: weight DMAs 2 stages ahead, projection
            # chunks interleaved between score tiles one stage ahead, PV one
            # head behind scores (also interleaved) ----
            tiles0, pg0 = make_proj(0, *ws[0])
            for _ in pg0:
                pass
            projs = {0: tiles0}
            pend = None
            pg = None
            for hp in range(HP):
                if hp + 2 < HP and hp + 2 not in ws and hp + 2 > 2:
                    ws[hp + 2] = dma_w(hp + 2)
                if hp + 1 < HP:
                    projs[hp + 1], pg = make_proj(hp + 1, *ws[hp + 1])
                    ws.pop(hp + 1)
                else:
                    pg = None
                kpTt, qpTt, vpmt = projs.pop(hp)
                pulled = 0
                pvg = None
                for r in range(2):
                    h = 2 * hp + r
                    pvg = pv_gen(*pend) if pend is not None else None
                    pts = []
                    for kt in range(KT):
                        pts.append(attn_tick(h, kpTt, qpTt, kt))
                        if pvg is not None and kt % 2 == 1:
                            next(pvg, None)
                        tick = r * KT + kt + 1
                        want = (tick * N_PROJ_PULLS) // (2 * KT)
                        while pg is not None and pulled < want:
                            if next(pg, StopIteration) is StopIteration:
                                pg = None
                                break
                            pulled += 1
                    pend = (h, pts, vpmt)
            for _ in pv_gen(*pend):
                pass

        # ---------------- tail: transpose ctx + output projection ----------
        with tc.tile_pool(name="ctxT", bufs=1) as ctp, \
             tc.tile_pool(name="ostage", bufs=2) as osp, \
             tc.tile_pool(name="tailps", bufs=2, space="PSUM") as tlp:
            # Wo loads into SBUF freed by the projection pools; the DMA can
            # start as soon as the last projection read retires
            wo_sb = ctp.tile([128, IO, HID], BF16, tag="wo")
            nc.sync.dma_start(wo_sb[:], wo_d.ap().rearrange("(io p) j -> p io j", p=128))
            ctxT = ctp.tile([128, IO, TQ], BF16)
            for jb in range(IO):
                ps = tlp.tile([128, 1024], BF16, tag="tpose")
                for qt in range(QT):
                    nc.tensor.matmul(
                        ps[:, qt * 128:(qt + 1) * 128],
                        ctxN[:, qt, jb * 128:(jb + 1) * 128], ident[:],
                        is_transpose=True, start=True, stop=True)
                nc.vector.tensor_copy(ctxT[:, jb, :], ps[:])
            for tt in range(QT):
                pso = big.tile([128, 1024], F32, tag="big")
                for half in range(2):
                    for jb in range(IO):
                        nc.tensor.matmul(
                            pso[:, half * 512:(half + 1) * 512],
                            ctxT[:, jb, tt * 128:(tt + 1) * 128],
                            wo_sb[:, jb, half * 512:(half + 1) * 512],
                            start=(jb == 0), stop=(jb == IO - 1))
                ost = osp.tile([128, 1024], F32, tag="ost")
                nc.vector.tensor_copy(ost[:], pso[:])
                nc.sync.dma_start(out_d.ap()[tt * 128:(tt + 1) * 128, :], ost[:])


def build():
    nc = bacc.Bacc("TRN2", target_bir_lowering=False, debug=False,
                   num_devices=N_CORES)
    qT_d = nc.dram_tensor("qT", [HID, TQ], BF16, kind="ExternalInput")
    kT_d = nc.dram_tensor("kT", [HID, T], BF16, kind="ExternalInput")
    vT_d = nc.dram_tensor("vT", [HID, T], BF16, kind="ExternalInput")
    bias_d = nc.dram_tensor("bias", [NCLS, T], F32, kind="ExternalInput")
    wqs_d = nc.dram_tensor("Wqs", [HP, 128, IO, 128], BF16, kind="ExternalInput")
    wks_d = nc.dram_tensor("Wks", [HP, 128, IO, 128], BF16, kind="ExternalInput")
    wvs_d = nc.dram_tensor("Wvs", [HP, 128, IO, 128], BF16, kind="ExternalInput")
    wo_d = nc.dram_tensor("Wo", [HID, HID], BF16, kind="ExternalInput")
    out_d = nc.dram_tensor("out", [TQ, HID], F32, kind="ExternalOutput")

    with tile.TileContext(nc) as tc:
        _emit(tc, qT_d, kT_d, vT_d, bias_d, wqs_d, wks_d, wvs_d, wo_d, out_d)
    nc.compile()
    return nc


_NC = None


def _get_nc():
    global _NC
    if _NC is None:
        _NC = build()
    return _NC


def _slice_weight(w):
    # [HID, HID] -> [HP, 128, IO, 128]: staged[hp, p, io, jj] = w[io*128+p, hp*128+jj]
    return np.ascontiguousarray(
        w.reshape(IO, 128, HP, 128).transpose(2, 1, 0, 3))


def kernel(**inputs):
    import ml_dtypes
    from concourse.bass_utils import run_bass_kernel_spmd

    bf16 = ml_dtypes.bfloat16
    q = np.asarray(inputs["q"], dtype=np.float32)
    k = np.asarray(inputs["k"], dtype=np.float32)
    v = np.asarray(inputs["v"], dtype=np.float32)
    pm = np.asarray(inputs["pad_mask"], dtype=np.float32)
    wqs = _slice_weight(
        (np.asarray(inputs["Wq"], dtype=np.float32) * (D ** -0.5)).astype(bf16))
    wks = _slice_weight(np.asarray(inputs["Wk"], dtype=np.float32).astype(bf16))
    wvs = _slice_weight(np.asarray(inputs["Wv"], dtype=np.float32).astype(bf16))
    wo = np.asarray(inputs["Wo"], dtype=np.float32).astype(bf16)

    # head h is masked by pad_mask[h % 4] (reference tiles the mask
    # head-major); shift keeps exp() inside fp8e4m3 range
    bias = (pm[0:NCLS] * NEG_INF + BIAS_SHIFT).astype(np.float32)

    kTs = [np.ascontiguousarray(k[b].T.astype(bf16)) for b in range(B)]
    vTs = [np.ascontiguousarray(v[b].T.astype(bf16)) for b in range(B)]

    in_maps = []
    for c in range(N_CORES):
        b, g = c // 2, c % 2
        qT = np.ascontiguousarray(q[b, g * TQ:(g + 1) * TQ, :].T.astype(bf16))
        in_maps.append({
            "qT": qT,
            "kT": kTs[b],
            "vT": vTs[b],
            "bias": bias,
            "Wqs": wqs, "Wks": wks, "Wvs": wvs, "Wo": wo,
        })
    res = run_bass_kernel_spmd(_get_nc(), in_maps, list(range(N_CORES))).results
    out = np.empty((B, T, HID), np.float32)
    for c in range(N_CORES):
        b, g = c // 2, c % 2
        out[b, g * TQ:(g + 1) * TQ] = res[c]["out"]
    return out
